# revision 5
# baseline (speedup 1.0000x reference)
"""ArcFace (AngularPenaltySMLoss) distributed Trainium2 kernel, v4.

Strategy (tensor-parallel over classes, per the sharding hint):
  - Shard W's C=100000 rows over 8 cores (12500 each).
  - Host: normalize x; pre-scale and cast x, W to fp8e4m3; lay both out
    chunk-contiguously so every DMA is 128 straight partition lines.
  - Device (SPMD, no collectives): per (chunk, b-tile) unit, fp8
    DoubleRow matmuls fill a [128, w] PSUM tile (si-outer / k-inner so
    column slices complete progressively). Each tile's columns are then
    consumed split across two engines, ratio ~61/39 so both hide under
    the PE stream:
      * cols [0:ca]  -> ACT: exp(2*raw) in place + accum_out (free-dim
        sum straight into an accumulator slot).
      * cols [ca:w]  -> DVE: Schraudolph bit-trick exp — tensor_scalar
        affine fp32->int16 (bits of bf16 exp), then one
        scalar_tensor_tensor fold-add over the bitcast-bf16 halves with
        accum_out (fp32).
  - The urgent transfers (x tiles + first W chunk) ride their own DMA
    queue so the 6.4MB W stream can't starve them.
  - Final per-bt reduce of the accumulator slots + [128, 8] DMA out.
  - Host: sum partials over cores, compute the tiny per-sample target /
    arccos / log path in f64, return the scalar loss.
"""

import sys

if "/opt/trn_rl_repo" not in sys.path:
    sys.path.insert(0, "/opt/trn_rl_repo")

import ml_dtypes
import numpy as np

import concourse.bass as bass
import concourse.mybir as mybir
from concourse import bacc
from concourse.bass_utils import run_bass_kernel_spmd
from concourse.tile import TileContext

B, C, D = 1024, 100000, 512
S_SCALE, MARGIN, EPS = 64.0, 0.5, 1e-7
N_CORES = 8
C_SHARD = C // N_CORES          # 12500
P = 128
KO = D // P                     # 4 k-chunks of 128
B_TILES = B // P                # 8
MM_N = 512                      # one matmul output <= one PSUM bank
N_WARM = 4                      # PE warm-up matmuls (bridge DMA fill + HAM)

WSCALE, XSCALE = 8.0, 4.0       # fp8 pre-scales (folded out via ACT_SCALE)
ACT_SCALE = S_SCALE / (WSCALE * XSCALE)   # 2.0

# Schraudolph bf16 exp bits: i16 = rint(A * raw + Badd); bitcast bf16.
# A = ACT_SCALE * 2^7/ln2; Badd = 127*2^7 - C with C calibrated to zero
# the mean relative error for s*logit ~ N(0, 1.28).
SCH_A = ACT_SCALE * 184.66496580927726
SCH_B = 16256.0 - 7.4

# (width, act_cols): per-chunk split of columns between ACT-exp and
# DVE-Schraudolph, balancing measured instruction costs:
#   ACT = 0.833*ca + 576ns   DVE = 1.45*cs + 444ns   PE fill = 0.84*w
CHUNK_SPEC = [
    (512, 312),
    (1748, 1052),
    (2048, 1244),
    (2048, 1244),
    (2048, 1244),
    (2048, 1244),
    (2048, 1244),
]
assert sum(w for w, _ in CHUNK_SPEC) == C_SHARD
N_CHUNKS = len(CHUNK_SPEC)

LAST_RESULT = None
_NC_CACHE = None


def _build_bass():
    nc = bacc.Bacc("TRN2")
    xnt = nc.declare_dram_parameter("xnt", [P, KO * B], mybir.dt.float8e4, isOutput=False)
    wt = nc.declare_dram_parameter("wt", [P, KO * C_SHARD], mybir.dt.float8e4, isOutput=False)
    out = nc.declare_dram_parameter("out", [P, B_TILES], mybir.dt.float32, isOutput=True)

    fp8 = mybir.dt.float8e4
    f32 = mybir.dt.float32
    bf16 = mybir.dt.bfloat16
    i16 = mybir.dt.int16
    DR = mybir.MatmulPerfMode.DoubleRow
    EXP = mybir.ActivationFunctionType.Exp

    with TileContext(nc) as tc:
        with (
            tc.tile_pool(name="xp", bufs=1) as xp,
            tc.tile_pool(name="wp", bufs=1) as wp,
            tc.tile_pool(name="ip", bufs=3) as ip,
            tc.tile_pool(name="fp", bufs=2) as fpool,
            tc.tile_pool(name="ac", bufs=1) as ac,
            tc.tile_pool(name="ps", bufs=2, space="PSUM") as psp,
        ):
            # urgent queue (sync): x tiles + first W chunk
            xa = xp.tile([P, 2, B], fp8)
            xb = xp.tile([P, 2, B], fp8)
            nc.sync.dma_start(xa[:], xnt[:, : 2 * B])
            nc.sync.dma_start(xb[:], xnt[:, 2 * B :])

            wts = []
            c0 = 0
            for ci, (cw, _) in enumerate(CHUNK_SPEC):
                t = wp.tile([P, KO, cw], fp8, tag=f"wt{ci}")
                q = nc.sync if ci == 0 else nc.scalar
                q.dma_start(t[:], wt[:, 4 * c0 : 4 * (c0 + cw)])
                wts.append(t)
                c0 += cw

            # ACT table warm-up: a tiny exp before any real work so the
            # ~2.7us PSEUDO_LOAD_ACT_FUNC_SET runs during the DMA fill.
            jt = xp.tile([P, 8], f32)
            nc.vector.memset(jt[:], 0.0)
            ja = xp.tile([P, 8], bf16)
            nc.scalar.activation(ja[:], jt[:], EXP)

            # PE warm-up: bridge from engine start to the first
            # data-dependent matmul so HAM un-throttles (~3.4us window).
            wsrc = xp.tile([P, MM_N], fp8, tag="warm_src")
            nc.vector.memset(wsrc[:], 1)
            for _ in range(N_WARM):
                pw = psp.tile([P, 2048], f32, tag="ps")
                nc.tensor.matmul(
                    pw[:, :MM_N], wsrc[:, :P], wsrc[:], start=True, stop=True
                )

            # accumulator: 2 slots (ACT half, DVE half) per (bt, chunk)
            acc = ac.tile([P, B_TILES, 2 * N_CHUNKS], f32)
            out_sb = ac.tile([P, B_TILES], f32)

            for ci, (cw, ca) in enumerate(CHUNK_SPEC):
                wt_t = wts[ci]
                cs = cw - ca
                h = cs // 2
                n_sub = (cw + MM_N - 1) // MM_N
                for bt in range(B_TILES):
                    ps = psp.tile([P, 2048], f32, tag="ps")
                    # si-outer / k-inner: column slices complete
                    # progressively so consumers can start early.
                    for si in range(n_sub):
                        s0 = si * MM_N
                        sw = min(MM_N, cw - s0)
                        for k in (0, 2):
                            nc.tensor.matmul(
                                ps[:, s0 : s0 + sw],
                                (xa if k == 0 else xb)[:, :, bt * P : (bt + 1) * P],
                                wt_t[:, k : k + 2, s0 : s0 + sw],
                                start=(k == 0),
                                stop=(k == 2),
                                perf_mode=DR,
                            )
                    # ACT cols: exp in place + free-dim accumulate
                    nc.scalar.activation(
                        ps[:, :ca],
                        ps[:, :ca],
                        EXP,
                        scale=ACT_SCALE,
                        accum_out=acc[:, bt, 2 * ci : 2 * ci + 1],
                    )
                    # DVE cols: Schraudolph exp bits + fused fold+accum
                    it = ip.tile([P, 1024], i16, tag="it")
                    nc.vector.tensor_scalar(
                        it[:, :cs],
                        ps[:, ca:cw],
                        SCH_A,
                        SCH_B,
                        mybir.AluOpType.mult,
                        mybir.AluOpType.add,
                    )
                    fo = fpool.tile([P, 512], bf16, tag="fo")
                    nc.vector.scalar_tensor_tensor(
                        fo[:, :h],
                        it[:, 0:h].bitcast(bf16),
                        1.0,
                        it[:, h:cs].bitcast(bf16),
                        mybir.AluOpType.mult,
                        mybir.AluOpType.add,
                        accum_out=acc[:, bt, 2 * ci + 1 : 2 * ci + 2],
                    )

            for bt in range(B_TILES):
                nc.vector.reduce_sum(
                    out_sb[:, bt : bt + 1],
                    acc[:, bt, :],
                    axis=mybir.AxisListType.X,
                )
            nc.sync.dma_start(out[:], out_sb[:])

    nc.compile()
    return nc


def _get_nc():
    global _NC_CACHE
    if _NC_CACHE is None:
        _NC_CACHE = _build_bass()
    return _NC_CACHE


def kernel(x: np.ndarray, labels: np.ndarray, W: np.ndarray) -> np.ndarray:
    global LAST_RESULT
    x = np.asarray(x, dtype=np.float32)
    W = np.asarray(W, dtype=np.float32)
    labels = np.asarray(labels)

    # ---- host prep (sharding glue) ----
    norms = np.maximum(np.sqrt((x.astype(np.float64) ** 2).sum(axis=1)), 1e-12)
    xn = (x / norms[:, None].astype(np.float32)).astype(np.float32)
    # xnt[p, ko, b] = xn[b, ko*128+p] * XSCALE
    xq = (
        np.ascontiguousarray(
            (xn.T * XSCALE).reshape(KO, P, B).transpose(1, 0, 2)
        )
        .astype(ml_dtypes.float8_e4m3)
        .reshape(P, KO * B)
    )

    in_maps = []
    for i in range(N_CORES):
        shard = W[i * C_SHARD : (i + 1) * C_SHARD]
        blocks = []
        c0 = 0
        for cw, _ in CHUNK_SPEC:
            blk = (shard[c0 : c0 + cw].T * WSCALE).reshape(KO, P, cw)
            blocks.append(blk.transpose(1, 0, 2).reshape(P, KO * cw))
            c0 += cw
        wt_q = np.concatenate(blocks, axis=1).astype(ml_dtypes.float8_e4m3)
        in_maps.append({"xnt": xq, "wt": np.ascontiguousarray(wt_q)})

    # ---- device: per-core partial sum over classes of exp(s*logit) ----
    nc = _get_nc()
    res = run_bass_kernel_spmd(nc, in_maps, core_ids=list(range(N_CORES)))
    LAST_RESULT = res

    # ---- host combine (the all-reduce + tiny per-sample tail) ----
    sumexp = np.zeros(B, dtype=np.float64)
    for i in range(N_CORES):
        part = res.results[i]["out"].astype(np.float64)  # [P, B_TILES]
        sumexp += part.T.reshape(B)                      # b = bt*128 + p

    target = np.einsum(
        "bd,bd->b", xn.astype(np.float64), W[labels].astype(np.float64)
    )
    tgt = np.clip(target, -1.0 + EPS, 1.0 - EPS)
    numerator = S_SCALE * np.cos(np.arccos(tgt) + MARGIN)
    excl = sumexp - np.exp(S_SCALE * tgt)
    L = numerator - np.log(np.exp(numerator) + excl)
    return np.array(-L.mean(), dtype=np.float32)
